# revision 37
# baseline (speedup 1.0000x reference)
"""Causal multi-head attention (B=2, N=2048, C=768, H=12, D=64) on 8 trn2 cores.

Sharding: 8 cores = 2 batches x 4 head-blocks (3 heads each). Each core
computes q/k/v projections for its 3 heads, causal flash-attention, and a
partial output projection (its 192 columns of Wo). Host sums the 4 partials
per batch (the "all-reduce") and adds the bias during the gather.

Device dataflow (per core, fp32 data, matmuls in float32r):
  xT (768,2048) feature-major in SBUF
  QK^T proj  -> qkt_sb dim-major (Q_h,K_h share base partitions for matmul)
  V proj     -> v_sb tokens-major with a ones column (denominator trick)
  S^T blocks (k in partition, q free) -> exp (scale=1/8 folded) -> causal
  mask via affine_select -> P^T @ V_aug accumulated over k -> O^T (+Z row 64)
  normalize O^T by 1/Z (partition broadcast + approx reciprocal)
  Y^T partial = per-head Wo^T chain (768,2048), per-qblock -> DMA out
"""

import numpy as np

B, N, C = 2, 2048, 768
H, D = 12, 64
HL = 3            # heads per core
HD = HL * D       # 192
KC = C // 128     # 6 contraction chunks
NT = N // 512     # 4 query blocks
SCALE = D ** -0.5  # 0.125

_CACHED_NC = None


def _build_nc():
    global _CACHED_NC
    if _CACHED_NC is not None:
        return _CACHED_NC
    import concourse.mybir as mybir
    import concourse.tile as tile
    from concourse import bacc

    f32 = mybir.dt.float32
    f32r = mybir.dt.float32r
    Exp = mybir.ActivationFunctionType.Exp
    is_ge = mybir.AluOpType.is_ge

    nc = bacc.Bacc("TRN2", target_bir_lowering=False, debug=False,
                   enable_asserts=False, num_devices=8)
    xt_d = nc.dram_tensor("xt", [C, N], f32r, kind="ExternalInput").ap()
    wqk_d = nc.dram_tensor("wqk", [C, 512], f32r, kind="ExternalInput").ap()
    wv_d = nc.dram_tensor("wv", [C, 256], f32r, kind="ExternalInput").ap()
    wo_d = nc.dram_tensor("wo", [HD, C], f32r, kind="ExternalInput").ap()
    yt_d = nc.dram_tensor("yt", [C, N], f32, kind="ExternalOutput").ap()

    # per-head (chunk, base-partition) in qkt_sb; Q_h and K_h share base.
    QLOC = [(0, 0), (0, 64), (2, 0)]
    KLOC = [(1, 0), (1, 64), (3, 0)]

    with tile.TileContext(nc) as tc:
        with (
            tc.tile_pool(name="wpool", bufs=1) as wpool,
            tc.tile_pool(name="qkpool", bufs=1) as qkpool,
            tc.tile_pool(name="vpool", bufs=1) as vpool,
            tc.tile_pool(name="opool", bufs=2) as opool,
            tc.tile_pool(name="ppsum", bufs=3, space="PSUM") as ppsum,
            tc.tile_pool(name="stpsum", bufs=2, space="PSUM") as stpsum,
            tc.tile_pool(name="otpsum", bufs=1, space="PSUM") as otpsum,
        ):
            wqk_sb = wpool.tile([128, KC, 512], f32r)
            wv_sb = wpool.tile([128, KC, 256], f32r)
            wo_sb = wpool.tile([64, HL, C], f32r)
            qkt_sb = qkpool.tile([128, 4, N], f32r)
            v_sb = vpool.tile([128, HL * 16, 65], f32r)
            ones_sb = wpool.tile([128, 1], f32)

            xpool_cm = tc.tile_pool(name="xpool", bufs=1)
            xpool = xpool_cm.__enter__()
            xt_sb = xpool.tile([128, KC, N], f32r)

            # ---- loads: tokblock-0 essentials first, chunk-interleaved ----
            xt_r = xt_d.rearrange("(c p) n -> p c n", p=128)
            wqk_r = wqk_d.rearrange("(c p) m -> p c m", p=128)
            wv_r = wv_d.rearrange("(c p) m -> p c m", p=128)
            for cp in range(3):
                cs = slice(cp * 2, cp * 2 + 2)
                nc.gpsimd.dma_start(xt_sb[:, cs, 0:512], xt_r[:, cs, 0:512])
                nc.sync.dma_start(wqk_sb[:, cs, :], wqk_r[:, cs, :])
                nc.scalar.dma_start(wv_sb[:, cs, :], wv_r[:, cs, :])
            nc.vector.memset(ones_sb[:], 1.0)
            nc.vector.tensor_copy(v_sb[:, :, 64:65],
                                  ones_sb[:].to_broadcast([128, HL * 16, 1]))
            for t in range(1, NT):
                for cp in range(3):
                    cs = slice(cp * 2, cp * 2 + 2)
                    sl = slice(t * 512, (t + 1) * 512)
                    eng = (nc.scalar if t == 1 else
                           (nc.sync if cp % 2 == 0 else nc.gpsimd))
                    eng.dma_start(xt_sb[:, cs, sl], xt_r[:, cs, sl])
            nc.sync.dma_start(wo_sb[:], wo_d.rearrange("(h p) c -> p h c", p=64))

            # ---- projections, as independent psum-group units (PE fillers) ----
            def proj_units(t):
                ts_ = slice(t * 512, (t + 1) * 512)
                units = []

                def qk_unit(m):
                    def emit():
                        ps = ppsum.tile([128, 512], f32, tag="proj")
                        for c in range(KC):
                            nc.tensor.matmul(ps[:], wqk_sb[:, c, m * 128:(m + 1) * 128],
                                             xt_sb[:, c, ts_],
                                             start=(c == 0), stop=(c == KC - 1))
                        nc.vector.tensor_copy(qkt_sb[:, m, ts_], ps[:])
                    return emit

                def v_unit(q):
                    def emit():
                        tok = t * 512 + q * 128
                        i = t * 4 + q
                        ps = ppsum.tile([128, 256], f32, tag="proj")
                        for c in range(KC):
                            nc.tensor.matmul(ps[:], xt_sb[:, c, tok:tok + 128],
                                             wv_sb[:, c, :],
                                             start=(c == 0), stop=(c == KC - 1))
                        vsrc = ps[:, 0:HD].rearrange("p (h d) -> p h d", h=HL)
                        nc.vector.tensor_copy(v_sb[:, i::16, 0:64], vsrc)
                    return emit

                for m in (0, 1):
                    units.append(qk_unit(m))
                for q in range(4):
                    units.append(v_unit(q))
                for m in (2, 3):
                    units.append(qk_unit(m))
                return units

            def emit_proj(t):
                for u in proj_units(t):
                    u()

            def yt_units(j, o_j, use_act=False):
                qs = slice(j * 512, (j + 1) * 512)
                units = []

                def ob_unit(ob):
                    def emit():
                        yt_sb = ytpool.tile([128, 512], f32, tag="yt")
                        ps = ppsum.tile([128, 512], f32, tag="proj")
                        for h in range(HL):
                            nc.tensor.matmul(ps[:], wo_sb[:, h, ob * 128:(ob + 1) * 128],
                                             o_j[0:64, h, :],
                                             start=(h == 0), stop=(h == HL - 1))
                        if use_act and ob % 2 == 0:
                            nc.scalar.copy(yt_sb[:], ps[:])
                        else:
                            nc.vector.tensor_copy(yt_sb[:], ps[:])
                        eng = nc.sync if ob % 2 == 0 else nc.gpsimd
                        eng.dma_start(yt_d[ob * 128:(ob + 1) * 128, qs], yt_sb[:])
                    return emit

                return [ob_unit(ob) for ob in range(KC)]

            def emit_attention(j, ptpool, smpool, ytpool, fillers):
                qs = slice(j * 512, (j + 1) * 512)
                o_j = opool.tile([65, HL, 512], f32r, tag="oj")
                zpl = opool.tile([1, HL, 512], f32, tag="zpl", bufs=2)
                nslots = HL * 4 * (j + 1)
                cadence = max(1, nslots // (len(fillers) + 1))
                slot = [0]
                for h in range(HL):
                    qm, qp = QLOC[h]
                    km, kp = KLOC[h]
                    ot = otpsum.tile([65, 512], f32, tag="ot")
                    nkb = 4 * (j + 1)
                    ngr = 2 * (j + 1)
                    pts = []
                    for g in range(ngr):
                        st = stpsum.tile([128, 2, 512], f32, tag="st")
                        pt = ptpool.tile([128, 2, 512], f32r, tag="pt")
                        pts.append(pt)
                        for li in range(2):
                            kb = 2 * g + li
                            nc.tensor.matmul(
                                st[:, li, :],
                                qkt_sb[kp:kp + 64, km, kb * 128:(kb + 1) * 128],
                                qkt_sb[qp:qp + 64, qm, qs],
                                start=True, stop=True)
                        nc.scalar.activation(pt[:], st[:], Exp, scale=SCALE)
                        for li in range(2):
                            kb = 2 * g + li
                            if kb >= 4 * j:  # diagonal: mask the 128-wide band
                                di = kb - 4 * j
                                blk = pt[:, li, 128 * di:128 * (di + 1)]
                                nc.gpsimd.affine_select(
                                    blk, blk, pattern=[[1, 128]], compare_op=is_ge,
                                    fill=0.0, base=0, channel_multiplier=-1)
                    for kb in range(nkb):
                        pt = pts[kb // 2]
                        li = kb % 2
                        lo = 128 * (kb - 4 * j) if kb >= 4 * j else 0
                        nc.tensor.matmul(ot[:, lo:512], v_sb[:, h * 16 + kb, :],
                                         pt[:, li, lo:512],
                                         start=(kb == 0), stop=(kb == nkb - 1))
                        slot[0] += 1
                        if fillers and slot[0] % cadence == 0:
                            fillers.popleft()()
                    nc.vector.tensor_copy(o_j[:, h, :], ot[:])
                    # normalize this head immediately: o[0:64] *= 1/o[64]
                    nc.gpsimd.dma_start(zpl[0:1, h, :], o_j[64:65, h, :].bitcast(f32))
                    zbc = smpool.tile([64, 512], f32, tag="zbc")
                    nc.gpsimd.partition_broadcast(zbc[:], zpl[0:1, h, :])
                    nc.vector.reciprocal_approx_fast(zbc[:], zbc[:])
                    sl = o_j[0:64, h, :]
                    nc.vector.tensor_mul(sl, sl.bitcast(f32), zbc[:])

                return o_j

            from collections import deque
            emit_proj(0)
            bpools_cm = [
                tc.tile_pool(name="ptpool", bufs=8),
                tc.tile_pool(name="smpool", bufs=2),
                tc.tile_pool(name="ytpool", bufs=3),
            ]
            ptpool, smpool, ytpool = [cm.__enter__() for cm in bpools_cm]
            # process order: biggest blocks early (rich filler overlap),
            # smallest block last (shortest exp-chase tail).
            order = [0, 2, 3, 1]
            proj_needed = {0: [1, 2], 2: [3], 3: [], 1: []}
            prev_yt = []
            for j in order:
                fillers = deque(prev_yt)
                for t in proj_needed[j]:
                    fillers.extend(proj_units(t))
                o_j = emit_attention(j, ptpool, smpool, ytpool, fillers)
                while fillers:
                    fillers.popleft()()
                prev_yt = yt_units(j, o_j, use_act=(j == order[-1]))
            for u in prev_yt:
                u()

            for cm in reversed(bpools_cm):
                cm.__exit__(None, None, None)
            xpool_cm.__exit__(None, None, None)

    nc.compile()
    _CACHED_NC = nc
    return nc


def _make_in_maps(x, Wq, Wk, Wv, Wo):
    x = np.asarray(x, np.float32)
    Wq = np.asarray(Wq, np.float32)
    Wk = np.asarray(Wk, np.float32)
    Wv = np.asarray(Wv, np.float32)
    Wo = np.asarray(Wo, np.float32)
    z64 = np.zeros((C, 64), np.float32)
    in_maps = []
    for c in range(8):
        b, hb = divmod(c, 4)
        s = slice(hb * HD, (hb + 1) * HD)
        wq_s = Wq[s].T  # (768, 192)
        wk_s = Wk[s].T
        wqk = np.concatenate(
            [wq_s[:, 0:128], wk_s[:, 0:128], wq_s[:, 128:HD], z64,
             wk_s[:, 128:HD], z64], axis=1)  # (768, 512)
        in_maps.append({
            "xt": np.ascontiguousarray(x[b].T),
            "wqk": np.ascontiguousarray(wqk),
            "wv": np.ascontiguousarray(
                np.concatenate([Wv[s].T, z64], axis=1)),
            "wo": np.ascontiguousarray(Wo[:, s].T),
        })
    return in_maps


def _gather(results, bo):
    out = np.zeros((B, N, C), np.float32)
    for c in range(8):
        out[c // 4] += results[c]["yt"].T
    out += np.asarray(bo, np.float32)[None, None, :]
    return out


def kernel(x, Wq, Wk, Wv, Wo, bo):
    from concourse.bass_utils import run_bass_kernel_spmd
    nc = _build_nc()
    in_maps = _make_in_maps(x, Wq, Wk, Wv, Wo)
    try:
        res = run_bass_kernel_spmd(nc, in_maps, core_ids=list(range(8)))
    except ModuleNotFoundError:
        # BASS_TRACE set but this axon deployment lacks the NTFF hook module
        import os
        os.environ["BASS_NEVER_TRACE"] = "1"
        res = run_bass_kernel_spmd(nc, in_maps, core_ids=list(range(8)))
    return _gather(res.results, bo)


# revision 38
# speedup vs baseline: 1.0135x; 1.0135x over previous
"""Causal multi-head attention (B=2, N=2048, C=768, H=12, D=64) on 8 trn2 cores.

Sharding: 8 cores = 2 batches x 4 head-blocks (3 heads each). Each core
computes q/k/v projections for its 3 heads, causal flash-attention, and a
partial output projection (its 192 columns of Wo). Host sums the 4 partials
per batch (the "all-reduce") and adds the bias during the gather.

Device dataflow (per core, fp32 data, matmuls in float32r):
  xT (768,2048) feature-major in SBUF
  QK^T proj  -> qkt_sb dim-major (Q_h,K_h share base partitions for matmul)
  V proj     -> v_sb tokens-major with a ones column (denominator trick)
  S^T blocks (k in partition, q free) -> exp (scale=1/8 folded) -> causal
  mask via affine_select -> P^T @ V_aug accumulated over k -> O^T (+Z row 64)
  normalize O^T by 1/Z (partition broadcast + approx reciprocal)
  Y^T partial = per-head Wo^T chain (768,2048), per-qblock -> DMA out
"""

import numpy as np

B, N, C = 2, 2048, 768
H, D = 12, 64
HL = 3            # heads per core
HD = HL * D       # 192
KC = C // 128     # 6 contraction chunks
NT = N // 512     # 4 query blocks
SCALE = D ** -0.5  # 0.125

_CACHED_NC = None


def _build_nc():
    global _CACHED_NC
    if _CACHED_NC is not None:
        return _CACHED_NC
    import concourse.mybir as mybir
    import concourse.tile as tile
    from concourse import bacc

    f32 = mybir.dt.float32
    f32r = mybir.dt.float32r
    Exp = mybir.ActivationFunctionType.Exp
    is_ge = mybir.AluOpType.is_ge

    nc = bacc.Bacc("TRN2", target_bir_lowering=False, debug=False,
                   enable_asserts=False, num_devices=8)
    xt_d = nc.dram_tensor("xt", [C, N], f32r, kind="ExternalInput").ap()
    wqk_d = nc.dram_tensor("wqk", [C, 512], f32r, kind="ExternalInput").ap()
    wv_d = nc.dram_tensor("wv", [C, 256], f32r, kind="ExternalInput").ap()
    wo_d = nc.dram_tensor("wo", [HD, C], f32r, kind="ExternalInput").ap()
    yt_d = nc.dram_tensor("yt", [C, N], f32, kind="ExternalOutput").ap()

    # per-head (chunk, base-partition) in qkt_sb; Q_h and K_h share base.
    QLOC = [(0, 0), (0, 64), (2, 0)]
    KLOC = [(1, 0), (1, 64), (3, 0)]

    with tile.TileContext(nc) as tc:
        with (
            tc.tile_pool(name="wpool", bufs=1) as wpool,
            tc.tile_pool(name="qkpool", bufs=1) as qkpool,
            tc.tile_pool(name="vpool", bufs=1) as vpool,
            tc.tile_pool(name="opool", bufs=2) as opool,
            tc.tile_pool(name="ppsum", bufs=3, space="PSUM") as ppsum,
            tc.tile_pool(name="stpsum", bufs=2, space="PSUM") as stpsum,
            tc.tile_pool(name="otpsum", bufs=1, space="PSUM") as otpsum,
        ):
            wqk_sb = wpool.tile([128, KC, 512], f32r)
            wv_sb = wpool.tile([128, KC, 256], f32r)
            wo_sb = wpool.tile([64, HL, C], f32r)
            qkt_sb = qkpool.tile([128, 4, N], f32r)
            v_sb = vpool.tile([128, HL * 16, 65], f32r)
            ones_sb = wpool.tile([128, 1], f32)

            xpool_cm = tc.tile_pool(name="xpool", bufs=1)
            xpool = xpool_cm.__enter__()
            xt_sb = xpool.tile([128, KC, N], f32r)

            # ---- loads: tokblock-0 essentials first, chunk-interleaved ----
            xt_r = xt_d.rearrange("(c p) n -> p c n", p=128)
            wqk_r = wqk_d.rearrange("(c p) m -> p c m", p=128)
            wv_r = wv_d.rearrange("(c p) m -> p c m", p=128)
            for cp in range(3):
                cs = slice(cp * 2, cp * 2 + 2)
                nc.gpsimd.dma_start(xt_sb[:, cs, 0:512], xt_r[:, cs, 0:512])
                nc.sync.dma_start(wqk_sb[:, cs, :], wqk_r[:, cs, :])
                nc.scalar.dma_start(wv_sb[:, cs, :], wv_r[:, cs, :])
            nc.vector.memset(ones_sb[:], 1.0)
            nc.vector.tensor_copy(v_sb[:, :, 64:65],
                                  ones_sb[:].to_broadcast([128, HL * 16, 1]))
            for t in range(1, NT):
                for cp in range(3):
                    cs = slice(cp * 2, cp * 2 + 2)
                    sl = slice(t * 512, (t + 1) * 512)
                    eng = (nc.scalar if t == 1 else
                           (nc.sync if cp % 2 == 0 else nc.gpsimd))
                    eng.dma_start(xt_sb[:, cs, sl], xt_r[:, cs, sl])
            nc.sync.dma_start(wo_sb[:], wo_d.rearrange("(h p) c -> p h c", p=64))

            # ---- projections, as independent psum-group units (PE fillers) ----
            def proj_units(t):
                ts_ = slice(t * 512, (t + 1) * 512)
                units = []

                def qk_unit(m):
                    def emit():
                        ps = ppsum.tile([128, 512], f32, tag="proj")
                        for c in range(KC):
                            nc.tensor.matmul(ps[:], wqk_sb[:, c, m * 128:(m + 1) * 128],
                                             xt_sb[:, c, ts_],
                                             start=(c == 0), stop=(c == KC - 1))
                        nc.vector.tensor_copy(qkt_sb[:, m, ts_], ps[:])
                    return emit

                def v_unit(q):
                    def emit():
                        tok = t * 512 + q * 128
                        i = t * 4 + q
                        ps = ppsum.tile([128, 256], f32, tag="proj")
                        for c in range(KC):
                            nc.tensor.matmul(ps[:], xt_sb[:, c, tok:tok + 128],
                                             wv_sb[:, c, :],
                                             start=(c == 0), stop=(c == KC - 1))
                        vsrc = ps[:, 0:HD].rearrange("p (h d) -> p h d", h=HL)
                        nc.vector.tensor_copy(v_sb[:, i::16, 0:64], vsrc)
                    return emit

                for m in (0, 1):
                    units.append(qk_unit(m))
                for q in range(4):
                    units.append(v_unit(q))
                for m in (2, 3):
                    units.append(qk_unit(m))
                return units

            def emit_proj(t):
                for u in proj_units(t):
                    u()

            def yt_units(j, o_j, use_act=False):
                qs = slice(j * 512, (j + 1) * 512)
                units = []

                def ob_unit(ob):
                    def emit():
                        yt_sb = ytpool.tile([128, 512], f32, tag="yt")
                        ps = ppsum.tile([128, 512], f32, tag="proj")
                        for h in range(HL):
                            nc.tensor.matmul(ps[:], wo_sb[:, h, ob * 128:(ob + 1) * 128],
                                             o_j[0:64, h, :],
                                             start=(h == 0), stop=(h == HL - 1))
                        if use_act and ob % 2 == 0:
                            nc.scalar.copy(yt_sb[:], ps[:])
                        else:
                            nc.vector.tensor_copy(yt_sb[:], ps[:])
                        eng = nc.sync if ob % 2 == 0 else nc.gpsimd
                        eng.dma_start(yt_d[ob * 128:(ob + 1) * 128, qs], yt_sb[:])
                    return emit

                return [ob_unit(ob) for ob in range(KC)]

            def emit_attention(j, ptpool, smpool, ytpool, fillers):
                qs = slice(j * 512, (j + 1) * 512)
                o_j = opool.tile([65, HL, 512], f32r, tag="oj")
                zpl = opool.tile([1, HL, 512], f32, tag="zpl", bufs=2)
                nslots = HL * 4 * (j + 1)
                cadence = max(1, nslots // (len(fillers) + 1))
                slot = [0]
                for h in range(HL):
                    qm, qp = QLOC[h]
                    km, kp = KLOC[h]
                    ot = otpsum.tile([65, 512], f32, tag="ot")
                    nkb = 4 * (j + 1)
                    ngr = 2 * (j + 1)
                    pts = []
                    for g in range(ngr):
                        st = stpsum.tile([128, 2, 512], f32, tag="st")
                        pt = ptpool.tile([128, 2, 512], f32r, tag="pt")
                        pts.append(pt)
                        for li in range(2):
                            kb = 2 * g + li
                            nc.tensor.matmul(
                                st[:, li, :],
                                qkt_sb[kp:kp + 64, km, kb * 128:(kb + 1) * 128],
                                qkt_sb[qp:qp + 64, qm, qs],
                                start=True, stop=True)
                        nc.scalar.activation(pt[:], st[:], Exp, scale=SCALE)
                        for li in range(2):
                            kb = 2 * g + li
                            if kb >= 4 * j:  # diagonal: mask the 128-wide band
                                di = kb - 4 * j
                                blk = pt[:, li, 128 * di:128 * (di + 1)]
                                nc.gpsimd.affine_select(
                                    blk, blk, pattern=[[1, 128]], compare_op=is_ge,
                                    fill=0.0, base=0, channel_multiplier=-1)
                    for kb in range(nkb):
                        pt = pts[kb // 2]
                        li = kb % 2
                        lo = 128 * (kb - 4 * j) if kb >= 4 * j else 0
                        nc.tensor.matmul(ot[:, lo:512], v_sb[:, h * 16 + kb, :],
                                         pt[:, li, lo:512],
                                         start=(kb == 0), stop=(kb == nkb - 1))
                        slot[0] += 1
                        if fillers and slot[0] % cadence == 0:
                            fillers.popleft()()
                    nc.vector.tensor_copy(o_j[:, h, :], ot[:])
                    # normalize this head immediately: o[0:64] *= 1/o[64]
                    nc.sync.dma_start(zpl[0:1, h, :], o_j[64:65, h, :].bitcast(f32))
                    zbc = smpool.tile([64, 512], f32, tag="zbc")
                    nc.gpsimd.partition_broadcast(zbc[:], zpl[0:1, h, :])
                    nc.vector.reciprocal_approx_fast(zbc[:], zbc[:])
                    sl = o_j[0:64, h, :]
                    nc.vector.tensor_mul(sl, sl.bitcast(f32), zbc[:])

                return o_j

            from collections import deque
            emit_proj(0)
            bpools_cm = [
                tc.tile_pool(name="ptpool", bufs=8),
                tc.tile_pool(name="smpool", bufs=2),
                tc.tile_pool(name="ytpool", bufs=3),
            ]
            ptpool, smpool, ytpool = [cm.__enter__() for cm in bpools_cm]
            # process order: biggest blocks early (rich filler overlap),
            # smallest block last (shortest exp-chase tail).
            order = [0, 2, 3, 1]
            proj_needed = {0: [1, 2], 2: [3], 3: [], 1: []}
            prev_yt = []
            for j in order:
                fillers = deque(prev_yt)
                for t in proj_needed[j]:
                    fillers.extend(proj_units(t))
                o_j = emit_attention(j, ptpool, smpool, ytpool, fillers)
                while fillers:
                    fillers.popleft()()
                prev_yt = yt_units(j, o_j, use_act=(j == order[-1]))
            for u in prev_yt:
                u()

            for cm in reversed(bpools_cm):
                cm.__exit__(None, None, None)
            xpool_cm.__exit__(None, None, None)

    nc.compile()
    _CACHED_NC = nc
    return nc


def _make_in_maps(x, Wq, Wk, Wv, Wo):
    x = np.asarray(x, np.float32)
    Wq = np.asarray(Wq, np.float32)
    Wk = np.asarray(Wk, np.float32)
    Wv = np.asarray(Wv, np.float32)
    Wo = np.asarray(Wo, np.float32)
    z64 = np.zeros((C, 64), np.float32)
    in_maps = []
    for c in range(8):
        b, hb = divmod(c, 4)
        s = slice(hb * HD, (hb + 1) * HD)
        wq_s = Wq[s].T  # (768, 192)
        wk_s = Wk[s].T
        wqk = np.concatenate(
            [wq_s[:, 0:128], wk_s[:, 0:128], wq_s[:, 128:HD], z64,
             wk_s[:, 128:HD], z64], axis=1)  # (768, 512)
        in_maps.append({
            "xt": np.ascontiguousarray(x[b].T),
            "wqk": np.ascontiguousarray(wqk),
            "wv": np.ascontiguousarray(
                np.concatenate([Wv[s].T, z64], axis=1)),
            "wo": np.ascontiguousarray(Wo[:, s].T),
        })
    return in_maps


def _gather(results, bo):
    out = np.zeros((B, N, C), np.float32)
    for c in range(8):
        out[c // 4] += results[c]["yt"].T
    out += np.asarray(bo, np.float32)[None, None, :]
    return out


def kernel(x, Wq, Wk, Wv, Wo, bo):
    from concourse.bass_utils import run_bass_kernel_spmd
    nc = _build_nc()
    in_maps = _make_in_maps(x, Wq, Wk, Wv, Wo)
    try:
        res = run_bass_kernel_spmd(nc, in_maps, core_ids=list(range(8)))
    except ModuleNotFoundError:
        # BASS_TRACE set but this axon deployment lacks the NTFF hook module
        import os
        os.environ["BASS_NEVER_TRACE"] = "1"
        res = run_bass_kernel_spmd(nc, in_maps, core_ids=list(range(8)))
    return _gather(res.results, bo)
